# revision 1
# baseline (speedup 1.0000x reference)
"""Trainium2 Bass kernel for GCE-GNN LocalAggregator (gnn_message_passing).

Computes, for each batch b:
    h = embedding[inputs]                            # [N, D] gather
    e_k = leakyrelu((h * a_k) @ h.T, 0.2)            # k = 0..3
    alpha = softmax(where(adj == k+1, e_k, -inf))    # edge-type select
    out = alpha @ h

Sharding: data-parallel over batch B=512 across 8 cores (64 batches/core).
The embedding table is staged in bf16 (11MB) per core; only looked-up rows
are read from HBM.

Key structural ideas vs the naive mapping:
  * One indirect DMA per PAIR gathers 128 rows (one per partition) into the
    pair layout h_pair[(u,i), g, d] - 32 gathers instead of 64, bf16.
  * hT (d on partitions) comes from 32 PE transposes of whole pairs.
  * e_k is SYMMETRIC in (i,j) (e_k[i,j] = sum_d h_i h_j a_k), so ONLY the
    transposed attention matrix xT is ever materialized, selected straight
    from the e buffer with host-transposed one-hot masks ([(v,j), g, i]
    reading of the same bytes). No transpose of x, no untransposed x at all.
  * h_pair is padded with a ones column (the per-pair gather leaves the
    129-col layout contiguous per instruction), so the out-matmul's last
    column yields the softmax row sums for free - the whole untransposed
    selection/exp pipeline and its masks are gone.
  * Edge-type selection is multiply-by-one-hot + reduce over k (k is the
    INNERMOST e dimension via the matmul rhs AP dim order); no-edge entries
    are zeroed exactly by a post-exp edge-mask multiply.
  * exp(leakyrelu(x)) == max(exp(x), exp(0.2x)).
  * bf16 everywhere off-PSUM; chunks of 8 pairs pipeline across engines,
    with program order interleaved so the in-order PE queue never parks
    later-chunk transposes in front of ready e-matmuls.
"""

import os
import sys

import numpy as np

for _p in ("/opt/trn_rl_repo",):
    if _p not in sys.path and os.path.isdir(_p):
        sys.path.insert(0, _p)

import ml_dtypes

import concourse.bass as bass
import concourse.bacc as bacc
import concourse.tile as tile
from concourse import mybir
from concourse.bass_utils import run_bass_kernel_spmd

B, N, D, V = 512, 64, 128, 43098
NCORES = 8
BC = B // NCORES          # 64 local batches per core
NPAIR = BC // 2           # 32 pairs
ALPHA = 0.2
NEG_BIG = -1.0e9          # exp(NEG_BIG) == 0; stands in for -9e15
CHUNKS = [(0, 8), (8, 8), (16, 8), (24, 2), (26, 2), (28, 2), (30, 2)]  # (start pair, n pairs)
NCH = len(CHUNKS)

FP32 = mybir.dt.float32
BF16 = mybir.dt.bfloat16
I32 = mybir.dt.int32
AF = mybir.ActivationFunctionType
OP = mybir.AluOpType
AX = mybir.AxisListType

BF = ml_dtypes.bfloat16


def build_nc():
    nc = bacc.Bacc("TRN2", target_bir_lowering=False, debug=False)

    emb_d = nc.dram_tensor("emb", [V, D], BF16, kind="ExternalInput")
    idx_d = nc.dram_tensor("idx", [128, NPAIR], I32, kind="ExternalInput")
    ident_d = nc.dram_tensor("ident", [128, 128], BF16, kind="ExternalInput")
    mt_d = nc.dram_tensor("mt", [128, 4 * NPAIR * N], mybir.dt.uint8, kind="ExternalInput")
    attn_d = nc.dram_tensor("attnT", [D, 4], FP32, kind="ExternalInput")
    out_d = nc.dram_tensor("out", [BC, N, D], FP32, kind="ExternalOutput")

    with tile.TileContext(nc) as tc:
        with (
            tc.tile_pool(name="singles", bufs=1) as singles,
            tc.tile_pool(name="big", bufs=1) as big,
            tc.tile_pool(name="chnk", bufs=4) as chnk,
            tc.tile_pool(name="outp", bufs=4) as outp,
            tc.tile_pool(name="ps_t", bufs=2, space="PSUM") as ps_t,
            tc.tile_pool(name="ps_e", bufs=3, space="PSUM") as ps_e,
            tc.tile_pool(name="ps_o", bufs=3, space="PSUM") as ps_o,
        ):
            # ---- tiny inputs (idx first: the gathers gate on it) ----
            idx_sb = singles.tile([128, NPAIR], I32)
            nc.sync.dma_start(out=idx_sb[:, :], in_=idx_d[:, :])
            attn_sb = singles.tile([128, 4], FP32)
            nc.sync.dma_start(out=attn_sb[:, :], in_=attn_d[:, :])
            ident = singles.tile([128, 128], BF16)
            nc.sync.dma_start(out=ident[:, :], in_=ident_d[:, :])

            # transposed edge-type masks [p, k, g, i] uint8
            mt_sb = big.tile([128, 4, NPAIR, N], mybir.dt.uint8, tag="mt")
            nc.sync.dma_start(
                out=mt_sb[:, :, :, :],
                in_=mt_d.ap().rearrange("p (k g j) -> p k g j", k=4, j=N),
            )

            # h_pair[(u,i), g, d | 1], hT[d, g, (u,i)], S[d, k, g, (u,i)]
            hp = big.tile([128, NPAIR, D + 1], BF16, tag="hp")
            hT = big.tile([128, NPAIR, 128], BF16, tag="hT")
            S_all = big.tile([128, 4, NPAIR, 128], BF16, tag="S")

            # ones column for the row-sum trick
            nc.vector.memset(hp[:, :, D : D + 1], 1.0)

            flat = "p g j -> p (g j)"
            chunk_state = {}

            def emit_ingest(c):
                """Gather + transpose + S for pair-group c."""
                g0, ch = CHUNKS[c]
                gs = slice(g0, g0 + ch)
                for g in range(g0, g0 + ch):
                    nc.gpsimd.indirect_dma_start(
                        out=hp[:, g, 0:D],
                        out_offset=None,
                        in_=emb_d[:, :],
                        in_offset=bass.IndirectOffsetOnAxis(
                            ap=idx_sb[:, g : g + 1], axis=0
                        ),
                    )
                for g in range(g0, g0 + ch):
                    t_ps = ps_t.tile([128, 128], BF16, tag="t_ps")
                    nc.tensor.transpose(
                        out=t_ps[:, :], in_=hp[:, g, 0:D], identity=ident[:, :]
                    )
                    if g % 2 == 0:
                        nc.scalar.copy(out=hT[:, g, :], in_=t_ps[:, :])
                    else:
                        nc.vector.tensor_copy(out=hT[:, g, :], in_=t_ps[:, :])
                for k in range(4):
                    if k % 2 == 0:
                        nc.vector.tensor_scalar_mul(
                            out=S_all[:, k, gs, :].rearrange("p g q -> p (g q)"),
                            in0=hT[:, gs, :].rearrange("p g q -> p (g q)"),
                            scalar1=attn_sb[:, k : k + 1],
                        )
                    else:
                        nc.scalar.activation(
                            out=S_all[:, k, gs, :].rearrange("p g q -> p (g q)"),
                            in_=hT[:, gs, :].rearrange("p g q -> p (g q)"),
                            func=AF.Copy,
                            scale=attn_sb[:, k : k + 1],
                        )

            def emit_emm_select(c):
                """e matmuls + edge-type select + exp for chunk c (xT only)."""
                g0, ch = CHUNKS[c]
                gs = slice(g0, g0 + ch)
                tg = f"s{ch}"
                alT = chnk.tile([128, ch, N], BF16, tag=f"alT{tg}")
                xe3 = chnk.tile([128, ch, N], BF16, tag=f"xe3{tg}")
                xe4 = chnk.tile([128, ch, N], BF16, tag=f"xe4{tg}")
                xT = chnk.tile([128, ch, N], BF16, tag=f"xT{tg}")
                e_c = chnk.tile([128, 4, ch, N], BF16, tag=f"e{tg}")
                chunk_state[c] = xT

                nc.vector.memset(alT[:, :, :], NEG_BIG)

                # e matmuls (rhs streams (k, j) column order)
                for gl in range(ch):
                    g = g0 + gl
                    e_ps = ps_e.tile([128, 4, N], FP32, tag="e_ps")
                    for u in range(2):
                        nc.tensor.matmul(
                            out=e_ps[u * 64 : (u + 1) * 64, :, :],
                            lhsT=hT[:, g, u * 64 : (u + 1) * 64],
                            rhs=S_all[:, :, g, u * 64 : (u + 1) * 64],
                            start=True,
                            stop=True,
                        )
                    if gl % 2 == 0:
                        nc.scalar.copy(out=e_c[:, :, gl, :], in_=e_ps[:, :, :])
                    else:
                        nc.vector.tensor_copy(out=e_c[:, :, gl, :], in_=e_ps[:, :, :])

                # transposed select over the NEG background
                # (e_k symmetric => same e bytes serve the [(v,j), g, i] view)
                for k in range(4):
                    nc.vector.copy_predicated(
                        out=alT[:, :, :],
                        mask=mt_sb[:, k, gs, :],
                        data=e_c[:, k, :, :],
                    )

                # xT = exp(leakyrelu(.)) = max(exp(.), exp(0.2 .)); NEG
                # entries give exact 0 through exp
                nc.scalar.activation(
                    out=xe3[:, :, :].rearrange(flat),
                    in_=alT[:, :, :].rearrange(flat),
                    func=AF.Exp,
                )
                nc.scalar.activation(
                    out=xe4[:, :, :].rearrange(flat),
                    in_=alT[:, :, :].rearrange(flat),
                    func=AF.Exp,
                    scale=ALPHA,
                )
                nc.vector.tensor_tensor(
                    out=xT[:, :, :], in0=xe3[:, :, :], in1=xe4[:, :, :], op=OP.max
                )

            def emit_out(c):
                """out matmuls (ones column -> row sums) + scaled evac + DMA."""
                xT = chunk_state.pop(c)
                g0, ch = CHUNKS[c]
                for gl in range(ch):
                    g = g0 + gl
                    o_ps = ps_o.tile([128, D + 1], FP32, tag="o_ps")
                    for u in range(2):
                        nc.tensor.matmul(
                            out=o_ps[u * 64 : (u + 1) * 64, :],
                            lhsT=xT[u * 64 : (u + 1) * 64, gl, :],
                            rhs=hp[u * 64 : (u + 1) * 64, g, :],
                            start=True,
                            stop=True,
                        )
                    rinv1 = outp.tile([128, 1], FP32, tag="rinv1")
                    nc.vector.reciprocal(out=rinv1[:, :], in_=o_ps[:, D : D + 1])
                    o_sb = outp.tile([128, D], FP32, tag="o_sb")
                    nc.scalar.activation(
                        out=o_sb[:, :],
                        in_=o_ps[:, 0:D],
                        func=AF.Copy,
                        scale=rinv1[:, :],
                    )
                    nc.sync.dma_start(
                        out=out_d.ap().rearrange("b i d -> (b i) d")[
                            128 * g : 128 * (g + 1), :
                        ],
                        in_=o_sb[:, :],
                    )

            # Software pipeline. PE program order per group c is
            #   T(c), e-mm(c), out-mm(c-1)
            # so the in-order PE queue never parks a later group's
            # gather-gated transposes in front of ready e-matmuls, and
            # out(c-1)'s xT is ready by the time e-mm(c) finishes.
            for c in range(NCH):
                emit_ingest(c)
                if c >= 1:
                    emit_out(c - 1)
                emit_emm_select(c)
            emit_out(NCH - 1)
    nc.compile()
    return nc


_CACHE = {}


def _compiled():
    if "nc" not in _CACHE:
        _CACHE["nc"] = build_nc()
    return _CACHE["nc"]


def _shard_inputs(inputs, adj, embedding, attn_a):
    inputs = np.asarray(inputs)
    adj = np.asarray(adj)
    emb16 = np.ascontiguousarray(np.asarray(embedding, dtype=np.float32).astype(BF))
    attnT = np.ascontiguousarray(np.asarray(attn_a, dtype=np.float32).T)  # [D, 4]
    ident = np.ascontiguousarray(np.eye(128).astype(BF))
    in_maps = []
    for c in range(NCORES):
        sl = slice(c * BC, (c + 1) * BC)
        # idx[(u,i), g] = inputs[c*BC + 2g+u, i]
        idx = np.ascontiguousarray(
            inputs[sl].reshape(NPAIR, 2, N).transpose(1, 2, 0).reshape(128, NPAIR)
            .astype(np.int32)
        )
        adj_r = adj[sl].reshape(NPAIR, 2, N, N).astype(np.int32)  # [g, u, i, j]
        Bm = adj_r.transpose(1, 3, 0, 2).reshape(128, NPAIR, N)  # [(v,j), g, i]
        # transposed one-hot edge-type masks [p, k, g, i] uint8
        mt = np.ascontiguousarray(
            np.stack([Bm == k + 1 for k in range(4)], axis=1)
            .astype(np.uint8).reshape(128, 4 * NPAIR * N)
        )
        in_maps.append(dict(emb=emb16, idx=idx, ident=ident, mt=mt, attnT=attnT))
    return in_maps


def kernel(inputs, adj, mask_item, item, embedding, attn_a):
    in_maps = _shard_inputs(inputs, adj, embedding, attn_a)
    res = run_bass_kernel_spmd(
        _compiled(), in_maps, core_ids=list(range(NCORES))
    ).results
    out = np.concatenate([np.asarray(res[c]["out"]) for c in range(NCORES)], axis=0)
    return out.astype(np.float32)

